# revision 15
# baseline (speedup 1.0000x reference)
"""GQA attention block (dense_transformer) on 8 trn2 cores — v2.

Tensor-parallel by kv-group (8 q heads + 1 k + 1 v per core, 512 W_dense
columns). All tile pools are opened once with a static PSUM budget of
exactly 8 banks (qkv 2, scores 2, cps/aux 2, dense 2) so the Tile
scheduler can overlap QKV(b1) with attention(b0) and dense(b0) with
attention(b1), keeping the PE warm (HAM at 2.4 GHz).

QKV runs m-outer / k-inner over 256-column position blocks (one PSUM bank,
double buffered). Scores accumulate into a single [128,1024] 2-bank tile
per (ki,hh) with one merged exp; causal masking is a post-exp multiply by
a 0/1 lower-triangle constant (block-local). PV augments V with a ones
column so the softmax denominator falls out of the same matmul; the
denominators are gathered via SBUF->SBUF DMA into one [8,1024] tile,
inverted with a single Ln+Exp pass, and broadcast across partitions with a
one-hot selection matmul. The whole denominator path runs in bf16 (error
stays relative). PSUM->SBUF copies ride on DVE; ACT only does exp/Ln.
Output partials are written bf16; the host sums the 8 cores in f32.
"""
import numpy as np
import ml_dtypes
from contextlib import ExitStack

import bass_rust
import concourse.bass as bass
import concourse.mybir as mybir
from concourse import tile
from concourse.bass_utils import run_bass_kernel_spmd

dt = mybir.dt
bf16 = ml_dtypes.bfloat16

B, S, HID = 2, 1024, 4096
NKV, G, HD = 8, 8, 64
NPOS = B * S
INV = 0.125
NCORES = 8

# ---------------------------------------------------------------------------
# walrus in this container takes at most ONE sync-wait per instruction; Tile
# attaches several (tail drain especially). Split extras onto same-engine nops.
_orig_exit = tile.TileContext.__exit__


def _split_waits(nc):
    for bb in nc.m.functions[0].blocks:
        out, extra = [], 0
        for inst in bb.instructions:
            si = inst.sync_info
            if si is not None and len(si.on_wait) > 1:
                waits = list(si.on_wait)
                for w in waits[:-1]:
                    nop = mybir.InstNoOp(name=f"I-wsplit-{nc.next_id()}")
                    nop.engine = inst.engine
                    nop.sync_info = bass_rust.SyncInfo(on_wait=[w], on_update=[])
                    nc.register_instruction(nop, overwrite=True)
                    out.append(nop)
                    extra += 1
                inst.sync_info = bass_rust.SyncInfo(
                    on_wait=[waits[-1]], on_update=list(si.on_update)
                )
            out.append(inst)
        if extra:
            bb.instructions = out


def _patched_exit(self, exc_type, exc_val, exc_tb):
    r = _orig_exit(self, exc_type, exc_val, exc_tb)
    _split_waits(self.nc)
    return r


if tile.TileContext.__exit__ is not _patched_exit:
    tile.TileContext.__exit__ = _patched_exit
# ---------------------------------------------------------------------------

_CACHED_NC = None


def build_program():
    global _CACHED_NC
    if _CACHED_NC is not None:
        return _CACHED_NC
    nc = bass.Bass()
    hst_d = nc.declare_dram_parameter("hst", [32, 128, NPOS], dt.bfloat16, isOutput=False)
    wq_d = nc.declare_dram_parameter("wq", [32, 128, 640], dt.bfloat16, isOutput=False)
    wd_d = nc.declare_dram_parameter("wd", [4, 128, 4096], dt.bfloat16, isOutput=False)
    cst_d = nc.declare_dram_parameter("cst", [128, 2048], dt.bfloat16, isOutput=False)
    tri_d = nc.declare_dram_parameter("tri", [128, 128], dt.bfloat16, isOutput=False)
    ab_d = nc.declare_dram_parameter("ab", [128, 128], dt.float32, isOutput=False)
    idn_d = nc.declare_dram_parameter("idn", [128, 128], dt.bfloat16, isOutput=False)
    sel_d = nc.declare_dram_parameter("sel", [8, 512], dt.bfloat16, isOutput=False)
    outp_d = nc.declare_dram_parameter("outp", [32, 128, NPOS], dt.bfloat16, isOutput=True)

    AF = mybir.ActivationFunctionType

    with ExitStack() as ctx:
        tc = ctx.enter_context(tile.TileContext(nc))
        cpool = ctx.enter_context(tc.tile_pool(name="const", bufs=1))
        cst_sb = cpool.tile([128, 2048], dt.bfloat16)
        nc.sync.dma_start(cst_sb[:], cst_d[:])
        tri_sb = cpool.tile([128, 128], dt.bfloat16)
        nc.sync.dma_start(tri_sb[:], tri_d[:])
        ab_sb = cpool.tile([128, 128], dt.float32)
        nc.sync.dma_start(ab_sb[:], ab_d[:])
        idn_sb = cpool.tile([128, 128], dt.bfloat16)
        nc.sync.dma_start(idn_sb[:], idn_d[:])
        sel_sb = cpool.tile([8, 512], dt.bfloat16)
        nc.sync.dma_start(sel_sb[:], sel_d[:])

        hs_pool = ctx.enter_context(tc.tile_pool(name="hs", bufs=36))

        def hs_block(b, nblk, wq_sb=None):
            # first block interleaves its hs DMAs with the wq chunks so the
            # first QKV matmuls start within ~1us of kernel start
            tiles = []
            pcol = b * 1024 + nblk * 512
            for k in range(32):
                if wq_sb is not None:
                    nc.sync.dma_start(wq_sb[:, k * 640:(k + 1) * 640], wq_d[k])
                t = hs_pool.tile([128, 512], dt.bfloat16, tag="hs")
                nc.sync.dma_start(t[:], hst_d[k][:, pcol:pcol + 512])
                tiles.append(t)
            return tiles

        wq_sb = cpool.tile([128, 32 * 640], dt.bfloat16)
        hs_tiles = {(0, 0): hs_block(0, 0, wq_sb=wq_sb)}
        wd_sb = cpool.tile([128, 4 * 4096], dt.bfloat16)
        for kt in range(4):
            nc.sync.dma_start(wd_sb[:, kt * 4096:(kt + 1) * 4096], wd_d[kt])
        warm_sb = cpool.tile([128, 512], dt.bfloat16)
        nc.vector.memset(warm_sb[:], 0.5)

        raw_pool = ctx.enter_context(tc.tile_pool(name="raw", bufs=1))
        tmp_pool = ctx.enter_context(tc.tile_pool(name="tmp", bufs=1))
        pk_pool = ctx.enter_context(tc.tile_pool(name="pk", bufs=1))
        qp_pool = ctx.enter_context(tc.tile_pool(name="qp", bufs=2))
        kv_pool = ctx.enter_context(tc.tile_pool(name="kv", bufs=2))
        va_pool = ctx.enter_context(tc.tile_pool(name="va", bufs=2))
        exp_pool = ctx.enter_context(tc.tile_pool(name="exp", bufs=1))
        l_pool = ctx.enter_context(tc.tile_pool(name="l", bufs=1))
        rb_pool = ctx.enter_context(tc.tile_pool(name="rb", bufs=2))
        ctx_pool = ctx.enter_context(tc.tile_pool(name="ctx", bufs=1))
        dout_pool = ctx.enter_context(tc.tile_pool(name="dout", bufs=2))

        # static PSUM budget: qkv 2 + sc 2 + ctx0/ctx1 2 + dense 2 = 8 banks
        qkv_psum = ctx.enter_context(tc.tile_pool(name="qkvps", bufs=2, space="PSUM"))
        sc_psum = ctx.enter_context(tc.tile_pool(name="scps", bufs=1, space="PSUM"))
        at_psum = ctx.enter_context(tc.tile_pool(name="attps", bufs=1, space="PSUM"))
        d_psum = ctx.enter_context(tc.tile_pool(name="dps", bufs=2, space="PSUM"))

        # HAM warmup: the DMA prologue would otherwise leave the PE idle/cold
        # for its first ~25us (K=4/8 halves matmul throughput for ~3.4us after
        # any idle window).  A throwaway accumulation keeps the array hot.
        def warm_mms(n):
            # uses the scores slot: attention hasn't started during the
            # prologue, and this must not contend with the live qkv
            # accumulators (slot-wait ahead of them in the PE stream would
            # deadlock the engine).
            wps = sc_psum.tile([128, 512], dt.float32, tag="sc", name="warm")
            for i in range(n):
                nc.tensor.matmul(wps[:], warm_sb[:, 0:128], warm_sb[:],
                                 start=(i == 0), stop=(i == n - 1))

        for b in range(2):
            # ---------------- QKV projection + RoPE ----------------
            qp = [qp_pool.tile([128, 1024], dt.bfloat16, tag=f"qp{p}", name=f"qp{p}") for p in range(4)]
            kdup = kv_pool.tile([128, 1024], dt.bfloat16, tag="kdup")
            vt = kv_pool.tile([64, 1024], dt.bfloat16, tag="vt")
            for nblk in range(2):
                hst = hs_tiles.pop((b, nblk))
                # prefetch next block
                nxt = (b, nblk + 1) if nblk < 1 else (b + 1, 0)
                if nxt[0] < 2:
                    hs_tiles[nxt] = hs_block(*nxt)
                ncol = slice(nblk * 512, nblk * 512 + 512)
                raw = [raw_pool.tile([128, 512], dt.bfloat16, tag=f"raw{m}", name=f"raw{m}")
                       for m in range(5)]
                first = (b == 0 and nblk == 0)
                if first:
                    warm_mms(6)
                # m-groups of <=2 keep the live accumulator count at 2 banks
                # while k-inner order releases hs tiles early for prefetch
                for grp in ((0, 1), (2, 3), (4,)):
                    ps = {m: qkv_psum.tile([128, 512], dt.float32, tag="qkv",
                                           name=f"qkv{m}") for m in grp}
                    for k in range(32):
                        if first and grp == (0, 1):
                            # first pass is DMA-gated; keep the array hot
                            warm_mms(2)
                        for m in grp:
                            nc.tensor.matmul(
                                ps[m][:],
                                wq_sb[:, k * 640 + m * 128: k * 640 + (m + 1) * 128],
                                hst[k][:],
                                start=(k == 0), stop=(k == 31),
                            )
                    for m in grp:
                        nc.vector.tensor_copy(raw[m][:], ps[m][:])
                Cs = cst_sb[:, nblk * 512:(nblk + 1) * 512]
                Ss = cst_sb[:, 1024 + nblk * 512: 1024 + (nblk + 1) * 512]
                for grp in range(2):
                    A, Bb = raw[grp * 2], raw[grp * 2 + 1]
                    P1 = tmp_pool.tile([128, 512], dt.bfloat16, tag="P1")
                    P2 = tmp_pool.tile([128, 512], dt.bfloat16, tag="P2")
                    P3 = tmp_pool.tile([128, 512], dt.bfloat16, tag="P3")
                    P4 = tmp_pool.tile([128, 512], dt.bfloat16, tag="P4")
                    nc.vector.tensor_mul(P1[:], A[:], Cs)
                    nc.vector.tensor_mul(P2[:], Bb[:], Ss)
                    nc.vector.tensor_mul(P3[:], Bb[:], Cs)
                    nc.vector.tensor_mul(P4[:], A[:], Ss)
                    for i in range(4):
                        h = grp * 4 + i
                        pr, sub = h // 2, h % 2
                        sl = slice(32 * i, 32 * i + 32)
                        nc.vector.tensor_sub(
                            qp[pr][sub * 64: sub * 64 + 32, ncol], P1[sl, :], P2[sl, :])
                        nc.vector.tensor_add(
                            qp[pr][sub * 64 + 32: sub * 64 + 64, ncol], P3[sl, :], P4[sl, :])
                kvr = raw[4]
                pk1 = pk_pool.tile([32, 512], dt.bfloat16, tag="pk1")
                pk2 = pk_pool.tile([32, 512], dt.bfloat16, tag="pk2")
                pk3 = pk_pool.tile([32, 512], dt.bfloat16, tag="pk3")
                pk4 = pk_pool.tile([32, 512], dt.bfloat16, tag="pk4")
                nc.vector.tensor_mul(pk1[:], kvr[0:32, :], Cs[0:32, :])
                nc.vector.tensor_mul(pk2[:], kvr[32:64, :], Ss[32:64, :])
                nc.vector.tensor_mul(pk3[:], kvr[32:64, :], Cs[32:64, :])
                nc.vector.tensor_mul(pk4[:], kvr[0:32, :], Ss[0:32, :])
                nc.vector.tensor_sub(kdup[0:32, ncol], pk1[:], pk2[:])
                nc.vector.tensor_add(kdup[32:64, ncol], pk3[:], pk4[:])
                nc.vector.tensor_copy(kdup[64:128, ncol], kdup[0:64, ncol])
                nc.vector.tensor_copy(vt[:, ncol], kvr[64:128, :])

            # ---------------- V transpose + ones column ----------------
            va = va_pool.tile([128, 8 * 72], dt.bfloat16, tag="va")
            for ki in range(8):
                vps = at_psum.tile([128, 64], dt.bfloat16, tag="ctx1")
                nc.tensor.transpose(vps[:], vt[0:64, ki * 128:(ki + 1) * 128],
                                    idn_sb[0:64, 0:64])
                nc.scalar.copy(va[:, ki * 72: ki * 72 + 64], vps[:])
                nc.vector.memset(va[:, ki * 72 + 64: ki * 72 + 65], 1.0)

            # ---------------- attention ----------------
            L8 = l_pool.tile([8, 1024], dt.bfloat16, tag="L8")
            ctxu_tiles = []
            for pr in range(4):
                ctxu = ctx_pool.tile([128, 1024], dt.bfloat16, tag=f"ctxu{pr}")
                ctxu_tiles.append(ctxu)
                ets = {}
                for half in range(2):
                    # scores + exp for ki in this half
                    for ki in range(half * 4, half * 4 + 4):
                        base = ki * 128
                        w = 1024 - base
                        for hh in range(2):
                            et = exp_pool.tile([128, w], dt.bfloat16,
                                               tag=f"e{hh}_{ki}", name=f"e{hh}_{ki}")
                            ets[(hh, ki)] = et
                            sc = sc_psum.tile([128, 1024], dt.float32, tag="sc")
                            cuts = [base] + [c for c in (512,) if c > base] + [1024]
                            for c0, c1 in zip(cuts[:-1], cuts[1:]):
                                nc.tensor.matmul(
                                    sc[:, c0:c1],
                                    kdup[hh * 64:(hh + 1) * 64, base:base + 128],
                                    qp[pr][hh * 64:(hh + 1) * 64, c0:c1],
                                    start=True, stop=True,
                                )
                            abc = b * 64 + ki * 8 + pr * 2 + hh
                            nc.scalar.activation(
                                et[:, 0:w], sc[:, base:1024], AF.Exp,
                                bias=ab_sb[:, abc:abc + 1], scale=INV)
                            # causal mask: zero the upper triangle of the
                            # diagonal 128-block (block-local, constant)
                            nc.vector.tensor_mul(
                                et[:, 0:128], et[:, 0:128], tri_sb[:])
                    # PV for q-columns in this half, batched in 256-wide pairs
                    # of q-tiles.  The pair's diagonal N=128 matmul goes FIRST
                    # with start=True (clears the bank's has_written bits);
                    # the N=256 matmuls then overwrite-then-accumulate both
                    # regions with start=False.
                    cps = [at_psum.tile([65, 512], dt.float32, tag=f"ctx{j}", name=f"ctx{j}")
                           for j in range(2)]
                    for kp in (half * 2, half * 2 + 1):
                        loc = (kp - half * 2) * 256
                        dki = 2 * kp + 1
                        for hh in range(2):
                            nc.tensor.matmul(
                                cps[hh][:, loc + 128:loc + 256],
                                va[:, dki * 72: dki * 72 + 65],
                                ets[(hh, dki)][:, 0:128],
                                start=True, stop=False,
                            )
                        for ki in range(2 * kp + 1):
                            for hh in range(2):
                                nc.tensor.matmul(
                                    cps[hh][:, loc:loc + 256],
                                    va[:, ki * 72: ki * 72 + 65],
                                    ets[(hh, ki)][:, 256 * kp - 128 * ki: 256 * kp - 128 * ki + 256],
                                    start=False, stop=(ki == 2 * kp),
                                )
                    # evacuate un-normalized context + denominators
                    hsl = slice(half * 512, half * 512 + 512)
                    for hh in range(2):
                        nc.vector.tensor_copy(
                            ctxu[hh * 64:(hh + 1) * 64, hsl], cps[hh][0:64, :])
                        # engines can't shift partition start mod 32; bounce
                        # the denominator row via a p0 tile + SBUF->SBUF DMA.
                        lrow = rb_pool.tile([1, 512], dt.bfloat16, tag="lrow")
                        nc.vector.tensor_copy(lrow[:], cps[hh][64:65, :])
                        idx = pr * 2 + hh
                        nc.sync.dma_start(L8[idx:idx + 1, hsl], lrow[:])

            # batched denominator: r = 1/l for all 8 (pr,hh) rows at once
            LL = l_pool.tile([8, 1024], dt.float32, tag="LL")
            nc.scalar.activation(LL[:], L8[:], AF.Ln)
            RR = l_pool.tile([8, 1024], dt.bfloat16, tag="RR")
            nc.scalar.activation(RR[:], LL[:], AF.Exp, scale=-1.0)
            # broadcast each r row across 64 partitions and normalize
            ctxt_tiles = []
            for pr in range(4):
                ctxt = ctx_pool.tile([128, 1024], dt.bfloat16, tag=f"ctxt{pr}")
                ctxt_tiles.append(ctxt)
                rbs = rb_pool.tile([128, 1024], dt.bfloat16, tag="rbs")
                for half in range(2):
                    hsl = slice(half * 512, half * 512 + 512)
                    rps = at_psum.tile([128, 512], dt.float32, tag="ctx0")
                    for hh in range(2):
                        idx = pr * 2 + hh
                        nc.tensor.matmul(rps[hh * 64:(hh + 1) * 64, :],
                                         sel_sb[:, idx * 64:(idx + 1) * 64],
                                         RR[:, hsl], start=True, stop=True)
                    nc.vector.tensor_copy(rbs[:, hsl], rps[:])
                nc.vector.tensor_mul(ctxt[:], ctxu_tiles[pr][:], rbs[:])

            # ---------------- dense partial -> DRAM ----------------
            for mt in range(32):
                dsb = dout_pool.tile([128, 1024], dt.bfloat16, tag="dsb")
                for n2 in range(2):
                    dps = d_psum.tile([128, 512], dt.float32, tag="d")
                    for kt in range(4):
                        nc.tensor.matmul(
                            dps[:],
                            wd_sb[:, kt * 4096 + mt * 128: kt * 4096 + (mt + 1) * 128],
                            ctxt_tiles[kt][:, n2 * 512:(n2 + 1) * 512],
                            start=(kt == 0), stop=(kt == 3),
                        )
                    nc.vector.tensor_copy(dsb[:, n2 * 512:(n2 + 1) * 512], dps[:])
                nc.sync.dma_start(
                    outp_d[mt][:, b * 1024: b * 1024 + 1024], dsb[:])

    _CACHED_NC = nc
    return nc


def host_prep(hidden_states, alibi, attention_mask, W_qkv, W_dense):
    hsT = np.ascontiguousarray(hidden_states.reshape(NPOS, HID).T).astype(bf16)
    hsT = hsT.reshape(32, 128, NPOS)

    j32 = np.arange(32)
    inv_freq = 1.0 / (10000.0 ** (2 * j32 / HD))
    t = np.arange(S, dtype=np.float64)
    fr = np.outer(inv_freq, t)                       # [32, S]
    cst = np.zeros((128, 2048), np.float32)
    cst[:, 0:1024] = np.tile(np.cos(fr), (4, 1))
    cst[:, 1024:2048] = np.tile(np.sin(fr), (4, 1))
    cst = cst.astype(bf16)

    # block-local causal 0/1 mask: tri[k,q] = 1 if k <= q
    tri = np.triu(np.ones((128, 128), np.float32)).astype(bf16)

    al = alibi.reshape(B, NKV * G, S) * INV          # [B, 64, S]

    perm = []
    for i in range(4):
        perm += [i * 64 + d for d in range(32)]
    for i in range(4):
        perm += [i * 64 + 32 + d for d in range(32)]
    for i in range(4, 8):
        perm += [i * 64 + d for d in range(32)]
    for i in range(4, 8):
        perm += [i * 64 + 32 + d for d in range(32)]
    perm += [512 + d for d in range(64)] + [576 + d for d in range(64)]
    perm = np.array(perm)

    idn = np.eye(128, dtype=np.float32).astype(bf16)
    sel = np.kron(np.eye(8, dtype=np.float32), np.ones((1, 64), np.float32)).astype(bf16)
    in_maps = []
    for c in range(NCORES):
        Wg = W_qkv[c * 640:(c + 1) * 640][perm]       # [640, 4096]
        wq = np.ascontiguousarray(Wg.T).astype(bf16).reshape(32, 128, 640)
        Wd = W_dense[:, c * 512:(c + 1) * 512]        # [4096, 512]
        wd = np.ascontiguousarray(Wd.T).astype(bf16).reshape(4, 128, 4096)
        ab = np.zeros((128, 128), np.float32)
        for b in range(2):
            for ki in range(8):
                for h in range(8):
                    ab[:, b * 64 + ki * 8 + h] = al[b, c * 8 + h,
                                                    ki * 128:(ki + 1) * 128]
        in_maps.append({
            "hst": hsT, "wq": wq, "wd": wd, "cst": cst,
            "tri": tri, "ab": ab, "idn": idn, "sel": sel,
        })
    return in_maps


def kernel(hidden_states, alibi, attention_mask, W_qkv, W_dense, _want_time=False):
    nc = build_program()
    in_maps = host_prep(np.asarray(hidden_states), np.asarray(alibi),
                        np.asarray(attention_mask), np.asarray(W_qkv),
                        np.asarray(W_dense))
    res = run_bass_kernel_spmd(nc, in_maps, list(range(NCORES)), trace=_want_time)
    acc = np.zeros((32, 128, NPOS), np.float32)
    for c in range(NCORES):
        acc += res.results[c]["outp"].astype(np.float32)
    out = acc.reshape(4096, NPOS).T.reshape(B, S, HID)
    if _want_time:
        return np.ascontiguousarray(out), res
    return np.ascontiguousarray(out)


# revision 16
# speedup vs baseline: 1.0505x; 1.0505x over previous
"""GQA attention block (dense_transformer) on 8 trn2 cores.

Tensor-parallel by kv-group (8 q heads + 1 k + 1 v per core, 512 W_dense
columns); hidden_states replicated, bf16 partial outputs summed on host in
f32.  All tile pools are opened once with a static PSUM budget of exactly
8 banks (qkv 2, scores 2, cps 2, dense 2) so the Tile scheduler can
overlap QKV(b1) with attention(b0) and dense(b0) with attention(b1),
keeping the PE warm (HAM at 2.4 GHz).  Throwaway warmup matmuls cover the
DMA-gated prologue.

QKV runs in m-groups of <=2 over 512-column position blocks (1-bank
accumulators, k-inner order releases hs tiles early for prefetch).
Scores accumulate into a single [128,1024] 2-bank tile per (ki,hh) with
one merged exp (alibi enters as the exp's per-partition bias); causal
masking is a post-exp multiply by a block-local 0/1 triangle constant.
PV batches q-tiles in 256-wide pairs: the pair's diagonal N=128 matmul
goes first with start=True to clear the bank's has_written bits, then the
N=256 matmuls overwrite-then-accumulate.  V is augmented with a ones
column so the softmax denominator falls out of the PV matmul; the
denominator rows are gathered via SBUF->SBUF DMA into one tile, inverted
with a batched Ln+Exp, and broadcast across partitions with a one-hot
selection matmul, all in bf16 (error stays relative).  PSUM->SBUF copies
ride on DVE; ACT only does exp/Ln.
"""
import numpy as np
import ml_dtypes
from contextlib import ExitStack

import bass_rust
import concourse.bass as bass
import concourse.mybir as mybir
from concourse import tile
from concourse.bass_utils import run_bass_kernel_spmd

dt = mybir.dt
bf16 = ml_dtypes.bfloat16

B, S, HID = 2, 1024, 4096
NKV, G, HD = 8, 8, 64
NPOS = B * S
INV = 0.125
NCORES = 8

# ---------------------------------------------------------------------------
# walrus in this container takes at most ONE sync-wait per instruction; Tile
# attaches several (tail drain especially). Split extras onto same-engine nops.
_orig_exit = tile.TileContext.__exit__


def _split_waits(nc):
    for bb in nc.m.functions[0].blocks:
        out, extra = [], 0
        for inst in bb.instructions:
            si = inst.sync_info
            if si is not None and len(si.on_wait) > 1:
                waits = list(si.on_wait)
                for w in waits[:-1]:
                    nop = mybir.InstNoOp(name=f"I-wsplit-{nc.next_id()}")
                    nop.engine = inst.engine
                    nop.sync_info = bass_rust.SyncInfo(on_wait=[w], on_update=[])
                    nc.register_instruction(nop, overwrite=True)
                    out.append(nop)
                    extra += 1
                inst.sync_info = bass_rust.SyncInfo(
                    on_wait=[waits[-1]], on_update=list(si.on_update)
                )
            out.append(inst)
        if extra:
            bb.instructions = out


def _patched_exit(self, exc_type, exc_val, exc_tb):
    r = _orig_exit(self, exc_type, exc_val, exc_tb)
    _split_waits(self.nc)
    return r


if tile.TileContext.__exit__ is not _patched_exit:
    tile.TileContext.__exit__ = _patched_exit
# ---------------------------------------------------------------------------

_CACHED_NC = None


def build_program():
    global _CACHED_NC
    if _CACHED_NC is not None:
        return _CACHED_NC
    nc = bass.Bass()
    hst_d = nc.declare_dram_parameter("hst", [32, 128, NPOS], dt.bfloat16, isOutput=False)
    wq_d = nc.declare_dram_parameter("wq", [32, 128, 640], dt.bfloat16, isOutput=False)
    wd_d = nc.declare_dram_parameter("wd", [4, 128, 4096], dt.bfloat16, isOutput=False)
    cst_d = nc.declare_dram_parameter("cst", [128, 2048], dt.bfloat16, isOutput=False)
    tri_d = nc.declare_dram_parameter("tri", [128, 128], dt.bfloat16, isOutput=False)
    ab_d = nc.declare_dram_parameter("ab", [128, 128], dt.float32, isOutput=False)
    idn_d = nc.declare_dram_parameter("idn", [128, 128], dt.bfloat16, isOutput=False)
    sel_d = nc.declare_dram_parameter("sel", [8, 512], dt.bfloat16, isOutput=False)
    outp_d = nc.declare_dram_parameter("outp", [32, 128, NPOS], dt.bfloat16, isOutput=True)

    AF = mybir.ActivationFunctionType

    with ExitStack() as ctx:
        tc = ctx.enter_context(tile.TileContext(nc))
        cpool = ctx.enter_context(tc.tile_pool(name="const", bufs=1))
        cst_sb = cpool.tile([128, 2048], dt.bfloat16)
        nc.sync.dma_start(cst_sb[:], cst_d[:])
        tri_sb = cpool.tile([128, 128], dt.bfloat16)
        nc.sync.dma_start(tri_sb[:], tri_d[:])
        ab_sb = cpool.tile([128, 128], dt.float32)
        nc.sync.dma_start(ab_sb[:], ab_d[:])
        idn_sb = cpool.tile([128, 128], dt.bfloat16)
        nc.sync.dma_start(idn_sb[:], idn_d[:])
        sel_sb = cpool.tile([8, 512], dt.bfloat16)
        nc.sync.dma_start(sel_sb[:], sel_d[:])

        hs_pool = ctx.enter_context(tc.tile_pool(name="hs", bufs=36))

        def hs_block(b, nblk, wq_sb=None):
            # first block interleaves its hs DMAs with the wq chunks so the
            # first QKV matmuls start within ~1us of kernel start
            tiles = []
            pcol = b * 1024 + nblk * 512
            for k in range(32):
                if wq_sb is not None:
                    nc.sync.dma_start(wq_sb[:, k * 640:(k + 1) * 640], wq_d[k])
                t = hs_pool.tile([128, 512], dt.bfloat16, tag="hs")
                nc.sync.dma_start(t[:], hst_d[k][:, pcol:pcol + 512])
                tiles.append(t)
            return tiles

        wq_sb = cpool.tile([128, 32 * 640], dt.bfloat16)
        hs_tiles = {(0, 0): hs_block(0, 0, wq_sb=wq_sb)}
        wd_sb = cpool.tile([128, 4 * 4096], dt.bfloat16)
        for kt in range(4):
            nc.sync.dma_start(wd_sb[:, kt * 4096:(kt + 1) * 4096], wd_d[kt])
        warm_sb = cpool.tile([128, 512], dt.bfloat16)
        nc.vector.memset(warm_sb[:], 0.5)

        raw_pool = ctx.enter_context(tc.tile_pool(name="raw", bufs=1))
        tmp_pool = ctx.enter_context(tc.tile_pool(name="tmp", bufs=1))
        pk_pool = ctx.enter_context(tc.tile_pool(name="pk", bufs=1))
        qp_pool = ctx.enter_context(tc.tile_pool(name="qp", bufs=2))
        kv_pool = ctx.enter_context(tc.tile_pool(name="kv", bufs=2))
        va_pool = ctx.enter_context(tc.tile_pool(name="va", bufs=2))
        exp_pool = ctx.enter_context(tc.tile_pool(name="exp", bufs=1))
        l_pool = ctx.enter_context(tc.tile_pool(name="l", bufs=1))
        rb_pool = ctx.enter_context(tc.tile_pool(name="rb", bufs=2))
        ctx_pool = ctx.enter_context(tc.tile_pool(name="ctx", bufs=1))
        dout_pool = ctx.enter_context(tc.tile_pool(name="dout", bufs=2))

        # static PSUM budget: qkv 2 + sc 2 + ctx0/ctx1 2 + dense 2 = 8 banks
        qkv_psum = ctx.enter_context(tc.tile_pool(name="qkvps", bufs=2, space="PSUM"))
        sc_psum = ctx.enter_context(tc.tile_pool(name="scps", bufs=1, space="PSUM"))
        at_psum = ctx.enter_context(tc.tile_pool(name="attps", bufs=1, space="PSUM"))
        d_psum = ctx.enter_context(tc.tile_pool(name="dps", bufs=2, space="PSUM"))

        # HAM warmup: the DMA prologue would otherwise leave the PE idle/cold
        # for its first ~25us (K=4/8 halves matmul throughput for ~3.4us after
        # any idle window).  A throwaway accumulation keeps the array hot.
        def warm_mms(n):
            # uses the scores slot: attention hasn't started during the
            # prologue, and this must not contend with the live qkv
            # accumulators (slot-wait ahead of them in the PE stream would
            # deadlock the engine).
            wps = sc_psum.tile([128, 512], dt.float32, tag="sc", name="warm")
            for i in range(n):
                nc.tensor.matmul(wps[:], warm_sb[:, 0:128], warm_sb[:],
                                 start=(i == 0), stop=(i == n - 1))

        for b in range(2):
            # ---------------- QKV projection + RoPE ----------------
            qp = [qp_pool.tile([128, 1024], dt.bfloat16, tag=f"qp{p}", name=f"qp{p}") for p in range(4)]
            kdup = kv_pool.tile([128, 1024], dt.bfloat16, tag="kdup")
            vt = kv_pool.tile([64, 1024], dt.bfloat16, tag="vt")
            for nblk in range(2):
                hst = hs_tiles.pop((b, nblk))
                # prefetch next block
                nxt = (b, nblk + 1) if nblk < 1 else (b + 1, 0)
                if nxt[0] < 2:
                    hs_tiles[nxt] = hs_block(*nxt)
                ncol = slice(nblk * 512, nblk * 512 + 512)
                raw = [raw_pool.tile([128, 512], dt.bfloat16, tag=f"raw{m}", name=f"raw{m}")
                       for m in range(5)]
                first = (b == 0 and nblk == 0)
                if first:
                    warm_mms(6)
                # m-groups of <=2 keep the live accumulator count at 2 banks
                # while k-inner order releases hs tiles early for prefetch
                for grp in ((0, 1), (2, 3), (4,)):
                    ps = {m: qkv_psum.tile([128, 512], dt.float32, tag="qkv",
                                           name=f"qkv{m}") for m in grp}
                    for k in range(32):
                        if first and grp == (0, 1):
                            # first pass is DMA-gated; keep the array hot
                            warm_mms(2)
                        for m in grp:
                            nc.tensor.matmul(
                                ps[m][:],
                                wq_sb[:, k * 640 + m * 128: k * 640 + (m + 1) * 128],
                                hst[k][:],
                                start=(k == 0), stop=(k == 31),
                            )
                    for m in grp:
                        nc.vector.tensor_copy(raw[m][:], ps[m][:])
                Cs = cst_sb[:, nblk * 512:(nblk + 1) * 512]
                Ss = cst_sb[:, 1024 + nblk * 512: 1024 + (nblk + 1) * 512]
                for grp in range(2):
                    A, Bb = raw[grp * 2], raw[grp * 2 + 1]
                    P1 = tmp_pool.tile([128, 512], dt.bfloat16, tag="P1")
                    P2 = tmp_pool.tile([128, 512], dt.bfloat16, tag="P2")
                    P3 = tmp_pool.tile([128, 512], dt.bfloat16, tag="P3")
                    P4 = tmp_pool.tile([128, 512], dt.bfloat16, tag="P4")
                    nc.vector.tensor_mul(P1[:], A[:], Cs)
                    nc.vector.tensor_mul(P2[:], Bb[:], Ss)
                    nc.vector.tensor_mul(P3[:], Bb[:], Cs)
                    nc.vector.tensor_mul(P4[:], A[:], Ss)
                    for i in range(4):
                        h = grp * 4 + i
                        pr, sub = h // 2, h % 2
                        sl = slice(32 * i, 32 * i + 32)
                        nc.vector.tensor_sub(
                            qp[pr][sub * 64: sub * 64 + 32, ncol], P1[sl, :], P2[sl, :])
                        nc.vector.tensor_add(
                            qp[pr][sub * 64 + 32: sub * 64 + 64, ncol], P3[sl, :], P4[sl, :])
                kvr = raw[4]
                pk1 = pk_pool.tile([32, 512], dt.bfloat16, tag="pk1")
                pk2 = pk_pool.tile([32, 512], dt.bfloat16, tag="pk2")
                pk3 = pk_pool.tile([32, 512], dt.bfloat16, tag="pk3")
                pk4 = pk_pool.tile([32, 512], dt.bfloat16, tag="pk4")
                nc.vector.tensor_mul(pk1[:], kvr[0:32, :], Cs[0:32, :])
                nc.vector.tensor_mul(pk2[:], kvr[32:64, :], Ss[32:64, :])
                nc.vector.tensor_mul(pk3[:], kvr[32:64, :], Cs[32:64, :])
                nc.vector.tensor_mul(pk4[:], kvr[0:32, :], Ss[0:32, :])
                nc.vector.tensor_sub(kdup[0:32, ncol], pk1[:], pk2[:])
                nc.vector.tensor_add(kdup[32:64, ncol], pk3[:], pk4[:])
                nc.vector.tensor_copy(kdup[64:128, ncol], kdup[0:64, ncol])
                nc.vector.tensor_copy(vt[:, ncol], kvr[64:128, :])

            # ---------------- V transpose + ones column ----------------
            va = va_pool.tile([128, 8 * 72], dt.bfloat16, tag="va")
            for ki in range(8):
                vps = at_psum.tile([128, 64], dt.bfloat16, tag="ctx1")
                nc.tensor.transpose(vps[:], vt[0:64, ki * 128:(ki + 1) * 128],
                                    idn_sb[0:64, 0:64])
                nc.scalar.copy(va[:, ki * 72: ki * 72 + 64], vps[:])
                nc.vector.memset(va[:, ki * 72 + 64: ki * 72 + 65], 1.0)

            # ---------------- attention ----------------
            L8 = l_pool.tile([8, 1024], dt.bfloat16, tag="L8")
            ctxu_tiles = []
            for pr in range(4):
                ctxu = ctx_pool.tile([128, 1024], dt.bfloat16, tag=f"ctxu{pr}")
                ctxu_tiles.append(ctxu)
                ets = {}
                for half in range(2):
                    # scores + exp for ki in this half
                    for ki in range(half * 4, half * 4 + 4):
                        base = ki * 128
                        w = 1024 - base
                        for hh in range(2):
                            et = exp_pool.tile([128, w], dt.bfloat16,
                                               tag=f"e{hh}_{ki}", name=f"e{hh}_{ki}")
                            ets[(hh, ki)] = et
                            sc = sc_psum.tile([128, 1024], dt.float32, tag="sc")
                            cuts = [base] + [c for c in (512,) if c > base] + [1024]
                            for c0, c1 in zip(cuts[:-1], cuts[1:]):
                                nc.tensor.matmul(
                                    sc[:, c0:c1],
                                    kdup[hh * 64:(hh + 1) * 64, base:base + 128],
                                    qp[pr][hh * 64:(hh + 1) * 64, c0:c1],
                                    start=True, stop=True,
                                )
                            abc = b * 64 + ki * 8 + pr * 2 + hh
                            nc.scalar.activation(
                                et[:, 0:w], sc[:, base:1024], AF.Exp,
                                bias=ab_sb[:, abc:abc + 1], scale=INV)
                            # causal mask: zero the upper triangle of the
                            # diagonal 128-block (block-local, constant)
                            nc.vector.tensor_mul(
                                et[:, 0:128], et[:, 0:128], tri_sb[:])
                    # PV for q-columns in this half, batched in 256-wide pairs
                    # of q-tiles.  The pair's diagonal N=128 matmul goes FIRST
                    # with start=True (clears the bank's has_written bits);
                    # the N=256 matmuls then overwrite-then-accumulate both
                    # regions with start=False.
                    cps = [at_psum.tile([65, 512], dt.float32, tag=f"ctx{j}", name=f"ctx{j}")
                           for j in range(2)]
                    for kp in (half * 2, half * 2 + 1):
                        loc = (kp - half * 2) * 256
                        dki = 2 * kp + 1
                        for hh in range(2):
                            nc.tensor.matmul(
                                cps[hh][:, loc + 128:loc + 256],
                                va[:, dki * 72: dki * 72 + 65],
                                ets[(hh, dki)][:, 0:128],
                                start=True, stop=False,
                            )
                        for ki in range(2 * kp + 1):
                            for hh in range(2):
                                nc.tensor.matmul(
                                    cps[hh][:, loc:loc + 256],
                                    va[:, ki * 72: ki * 72 + 65],
                                    ets[(hh, ki)][:, 256 * kp - 128 * ki: 256 * kp - 128 * ki + 256],
                                    start=False, stop=(ki == 2 * kp),
                                )
                    # evacuate un-normalized context + denominators
                    hsl = slice(half * 512, half * 512 + 512)
                    for hh in range(2):
                        nc.vector.tensor_copy(
                            ctxu[hh * 64:(hh + 1) * 64, hsl], cps[hh][0:64, :])
                        # engines can't shift partition start mod 32; bounce
                        # the denominator row via a p0 tile + SBUF->SBUF DMA.
                        lrow = rb_pool.tile([1, 512], dt.bfloat16, tag="lrow")
                        nc.vector.tensor_copy(lrow[:], cps[hh][64:65, :])
                        idx = pr * 2 + hh
                        nc.sync.dma_start(L8[idx:idx + 1, hsl], lrow[:])

            # batched denominator: r = 1/l for all 8 (pr,hh) rows at once
            LL = l_pool.tile([8, 1024], dt.float32, tag="LL")
            nc.scalar.activation(LL[:], L8[:], AF.Ln)
            RR = l_pool.tile([8, 1024], dt.bfloat16, tag="RR")
            nc.scalar.activation(RR[:], LL[:], AF.Exp, scale=-1.0)
            # broadcast each r row across 64 partitions and normalize
            ctxt_tiles = []
            for pr in range(4):
                ctxt = ctx_pool.tile([128, 1024], dt.bfloat16, tag=f"ctxt{pr}")
                ctxt_tiles.append(ctxt)
                rbs = rb_pool.tile([128, 1024], dt.bfloat16, tag="rbs")
                for half in range(2):
                    hsl = slice(half * 512, half * 512 + 512)
                    rps = at_psum.tile([128, 512], dt.float32, tag="ctx0")
                    for hh in range(2):
                        idx = pr * 2 + hh
                        nc.tensor.matmul(rps[hh * 64:(hh + 1) * 64, :],
                                         sel_sb[:, idx * 64:(idx + 1) * 64],
                                         RR[:, hsl], start=True, stop=True)
                    nc.vector.tensor_copy(rbs[:, hsl], rps[:])
                nc.vector.tensor_mul(ctxt[:], ctxu_tiles[pr][:], rbs[:])

            # ---------------- dense partial -> DRAM ----------------
            for mt in range(32):
                dsb = dout_pool.tile([128, 1024], dt.bfloat16, tag="dsb")
                for n2 in range(2):
                    dps = d_psum.tile([128, 512], dt.float32, tag="d")
                    for kt in range(4):
                        nc.tensor.matmul(
                            dps[:],
                            wd_sb[:, kt * 4096 + mt * 128: kt * 4096 + (mt + 1) * 128],
                            ctxt_tiles[kt][:, n2 * 512:(n2 + 1) * 512],
                            start=(kt == 0), stop=(kt == 3),
                        )
                    nc.vector.tensor_copy(dsb[:, n2 * 512:(n2 + 1) * 512], dps[:])
                nc.sync.dma_start(
                    outp_d[mt][:, b * 1024: b * 1024 + 1024], dsb[:])

    _CACHED_NC = nc
    return nc


def host_prep(hidden_states, alibi, attention_mask, W_qkv, W_dense):
    hsT = np.ascontiguousarray(hidden_states.reshape(NPOS, HID).T).astype(bf16)
    hsT = hsT.reshape(32, 128, NPOS)

    j32 = np.arange(32)
    inv_freq = 1.0 / (10000.0 ** (2 * j32 / HD))
    t = np.arange(S, dtype=np.float64)
    fr = np.outer(inv_freq, t)                       # [32, S]
    cst = np.zeros((128, 2048), np.float32)
    cst[:, 0:1024] = np.tile(np.cos(fr), (4, 1))
    cst[:, 1024:2048] = np.tile(np.sin(fr), (4, 1))
    cst = cst.astype(bf16)

    # block-local causal 0/1 mask: tri[k,q] = 1 if k <= q
    tri = np.triu(np.ones((128, 128), np.float32)).astype(bf16)

    al = alibi.reshape(B, NKV * G, S) * INV          # [B, 64, S]

    perm = []
    for i in range(4):
        perm += [i * 64 + d for d in range(32)]
    for i in range(4):
        perm += [i * 64 + 32 + d for d in range(32)]
    for i in range(4, 8):
        perm += [i * 64 + d for d in range(32)]
    for i in range(4, 8):
        perm += [i * 64 + 32 + d for d in range(32)]
    perm += [512 + d for d in range(64)] + [576 + d for d in range(64)]
    perm = np.array(perm)

    idn = np.eye(128, dtype=np.float32).astype(bf16)
    sel = np.kron(np.eye(8, dtype=np.float32), np.ones((1, 64), np.float32)).astype(bf16)
    in_maps = []
    for c in range(NCORES):
        Wg = W_qkv[c * 640:(c + 1) * 640][perm]       # [640, 4096]
        wq = np.ascontiguousarray(Wg.T).astype(bf16).reshape(32, 128, 640)
        Wd = W_dense[:, c * 512:(c + 1) * 512]        # [4096, 512]
        wd = np.ascontiguousarray(Wd.T).astype(bf16).reshape(4, 128, 4096)
        ab = np.zeros((128, 128), np.float32)
        for b in range(2):
            for ki in range(8):
                for h in range(8):
                    ab[:, b * 64 + ki * 8 + h] = al[b, c * 8 + h,
                                                    ki * 128:(ki + 1) * 128]
        in_maps.append({
            "hst": hsT, "wq": wq, "wd": wd, "cst": cst,
            "tri": tri, "ab": ab, "idn": idn, "sel": sel,
        })
    return in_maps


def kernel(hidden_states, alibi, attention_mask, W_qkv, W_dense, _want_time=False):
    nc = build_program()
    in_maps = host_prep(np.asarray(hidden_states), np.asarray(alibi),
                        np.asarray(attention_mask), np.asarray(W_qkv),
                        np.asarray(W_dense))
    res = run_bass_kernel_spmd(nc, in_maps, list(range(NCORES)), trace=_want_time)
    acc = np.zeros((32, 128, NPOS), np.float32)
    for c in range(NCORES):
        acc += res.results[c]["outp"].astype(np.float32)
    out = acc.reshape(4096, NPOS).T.reshape(B, S, HID)
    if _want_time:
        return np.ascontiguousarray(out), res
    return np.ascontiguousarray(out)
